# revision 17
# baseline (speedup 1.0000x reference)
"""Trainium2 Bass kernel for nn_ConvLayer: 3x3 conv (stride 1, pad 1) + per-channel offset.

Problem: x[32,64,56,56] (*) w[128,64,3,3] + offset[128,1,1] -> out[32,128,56,56], fp32.

Strategy (8 NeuronCores, data-parallel over batch, 4 images/core):
  - Conv as 9 shifted matmuls (one per 3x3 tap) accumulated in fp32 PSUM.
  - CIN=64 -> each tap is a contract-64 matmul = half the 128x128 PE array.
    Two images are processed CONCURRENTLY via 64x128 row tiling: image A's
    channels live in SBUF partitions 0-63 (PE tile (0,0)), image B's in
    partitions 64-127 (PE tile (64,0)). Each accumulates into its own PSUM
    bank, reaching full PE-array packing (measured 83ns per 399-col dual
    matmul slot = 100% of the 78.6 TF/s roofline).
  - All device-side tensors are fp16: x and weights are downcast on host
    (fp16 products accumulated in fp32 PSUM give ~5e-4 rel err vs the 2e-2
    gate), and the conv output is stored to HBM as fp16 and upcast to fp32
    on host. This halves HBM traffic (10.5 -> 5.1 MB/core) vs fp32.
  - Host pre-pads x to a 57-stride grid: ONE zero column is shared between
    consecutive rows (right-pad of row r == left-pad of row r+1), plus one
    zero row above and below. Every tap read is then a single contiguous
    shifted window. Weights are pre-transposed to [cin, tap, k] (lhsT
    layout) and duplicated into both partition halves.
  - Row-chunk sizes taper: a small first chunk (its input slice lands first
    so the matmul stream starts ASAP) and a small last chunk (so the final
    evict+store+completion tail after the last matmul is short).
  - Output columns are produced on the padded 57-wide grid; the PSUM->SBUF
    eviction (ScalarE for image A, VectorE for image B) compacts to the
    dense 56-wide grid, fuses the per-channel offset add, and downcasts to
    fp16. Each chunk's store is dispatched right after its eviction so the
    output stream never backs up at the end of the kernel.
"""

import numpy as np
from contextlib import ExitStack

import concourse.bass as bass
import concourse.tile as tile
from concourse import bacc, mybir
from concourse.bass_utils import run_bass_kernel_spmd

# Problem constants (hardcoded per contract).
B, CIN, HW, K = 32, 64, 56, 128
NCORES = 8
BPC = B // NCORES          # images per core
HP = HW + 1                # padded row stride: 57 (one shared pad col)
BASE = 1                   # element (row, col) lives at BASE + (row+1)*HP + col
NPAD = BASE + (HW + 2) * HP + 4   # 58 padded rows + tap-read slack: 3312
NOUT = HW * HW             # 3136
TAPS = 9
F16 = mybir.dt.float16
F32 = mybir.dt.float32

# Per-chunk output-row counts (sum 56). Small first chunk -> early start;
# small last chunk -> short tail. 7-row interior chunks (399 cols < 512
# fp32 = one PSUM bank).
RC = [4, 7, 7, 7, 7, 7, 7, 7, 3]
R0 = [sum(RC[:i]) for i in range(len(RC))]  # start row of each chunk
# Chunk groups per PSUM allocation (2 banks per chunk: one per image half).
# Single-chunk first/last groups; two groups in flight (<= 8 banks).
GROUPS = [(0,), (1, 2), (3, 4), (5, 6), (7,), (8,)]

_NC_CACHE = None


def _conv_kernel(ctx: ExitStack, tc: "tile.TileContext", out_ap, xp_ap, w2_ap, off_ap):
    nc = tc.nc
    singles = ctx.enter_context(tc.tile_pool(name="singles", bufs=1))
    xpool = ctx.enter_context(tc.tile_pool(name="xpool", bufs=2))
    opool = ctx.enter_context(tc.tile_pool(name="opool", bufs=2))
    psum = ctx.enter_context(tc.tile_pool(name="psum", bufs=8, space="PSUM"))

    # x-load slices (all on the Sync HWDGE ring, in consumption order).
    # Chunk c's tap reads end at (R0[c]+RC[c]+2)*57 + 2. A small first slice
    # gates chunk 0 so the first matmul starts early; the rest use large
    # per-partition runs for HBM read efficiency.
    gate = [(R0[c] + RC[c] + 2) * HP + 2 for c in range(len(RC))]
    xbounds = [0, gate[0], gate[2], gate[4], gate[6], NPAD]
    # chunks gated by x slices: c0 -> s0; c1,c2 -> s1; c3,c4 -> s2;
    # c5,c6 -> s3; c7,c8 -> s4.

    # Weights as lhsT [c, tap, k], duplicated across both partition halves.
    # One DMA, FIRST in the Sync ring's order: the x slices share that ring
    # and each SDMA engine drains its ring in order, so the weights complete
    # before the x stream — on a separate ring they would crawl at half rate
    # until ~12us (packet round-robin) and stall the first taps.
    # Split into partition halves: each half's descriptors live on a
    # disjoint set of 8 SDMA engines, so a single late-waking engine (a
    # ~2.5us straggler seen on ~1 device per run) only delays one PE half's
    # weights — the other half's matmul stream proceeds.
    w_sb = singles.tile([128, TAPS, K], F16)
    nc.sync.dma_start(w_sb[0:64], w2_ap[0:64])
    nc.sync.dma_start(w_sb[64:128], w2_ap[64:128])
    off_sb = singles.tile([128, 1], F32)
    nc.scalar.dma_start(off_sb[:], off_ap[:])

    # PE warmup: cheap bf16 matmuls on scratch keep TensorE busy through the
    # whole input-DMA head (~4us: body start ~7us to w+x landing ~9-12us,
    # worse when an SDMA engine wakes late). This keeps the HAM activity
    # window saturated (the clock gate opens 1.2 -> 2.4 GHz only after
    # sustained PE activity) and avoids PE-idle gaps on devices whose DMA
    # engines straggle. ~426ns each at the cold clock.
    scratch = singles.tile([128, 512], mybir.dt.bfloat16)
    nc.vector.memset(scratch[:], 0.0)
    ps_warm = psum.tile([128, 512], F32, tag="ps", name="ps_warm")
    for _ in range(7):
        nc.tensor.matmul(
            ps_warm[:], lhsT=scratch[0:64, 0:128], rhs=scratch[0:64, :],
            start=True, stop=True,
        )

    for pair in range(BPC // 2):
        b0 = 2 * pair
        # Both images of the pair side by side: [2, CIN, NPAD] -> [128, NPAD].
        x_t = xpool.tile([128, NPAD], F16, tag="x")
        xsrc = xp_ap[b0 : b0 + 2].rearrange("b c n -> (b c) n")
        for s in range(len(xbounds) - 1):
            lo, hi = xbounds[s], xbounds[s + 1]
            if s == 0 and pair == 0:
                # First slice gates the very first matmuls: split into
                # partition halves (disjoint SDMA-engine sets) so a
                # late-waking engine only stalls one PE half.
                nc.sync.dma_start(x_t[0:64, lo:hi], xsrc[0:64, lo:hi])
                nc.sync.dma_start(x_t[64:128, lo:hi], xsrc[64:128, lo:hi])
            else:
                nc.sync.dma_start(x_t[:, lo:hi], xsrc[:, lo:hi])
        o_sb = [
            opool.tile([128, NOUT], F16, tag="oA", name=f"oA_{pair}"),
            opool.tile([128, NOUT], F16, tag="oB", name=f"oB_{pair}"),
        ]

        for grp in GROUPS:
            ps = {}
            for half in (0, 1):
                for c in grp:
                    ps[(half, c)] = psum.tile(
                        [128, RC[c] * HP], F32, tag="ps",
                        name=f"ps_{pair}_{half}_{c}",
                    )
            for t in range(TAPS):
                kh, kw = divmod(t, 3)
                o = kh * HP + kw
                st, sp = (t == 0), (t == TAPS - 1)
                for half in (0, 1):
                    lo, hi = 64 * half, 64 * half + 64
                    for c in grp:
                        base = R0[c] * HP + o
                        nc.tensor.matmul(
                            ps[(half, c)][:],
                            lhsT=w_sb[lo:hi, t, :],
                            rhs=x_t[lo:hi, base : base + RC[c] * HP],
                            start=st,
                            stop=sp,
                        )
            # Evict: compact 57-stride padded rows to 56-wide dense rows, add
            # the per-channel offset, downcast to fp16. Image A on ScalarE,
            # image B on VectorE (they hit different PSUM banks in parallel).
            # Each chunk's store is dispatched right after its eviction so the
            # output stream never builds an end-of-kernel backlog: the final
            # chunk's store is the only transfer left after the last matmul.
            # Image A rides the Scalar HWDGE ring, image B the Sync ring.
            for c in grp:
                last = c == len(RC) - 1
                lo_col, hi_col = R0[c] * HW, (R0[c] + RC[c]) * HW
                pa = ps[(0, c)].rearrange("p (r x) -> p r x", x=HP)[:, :, 0:HW]
                oa = o_sb[0][:, lo_col:hi_col].rearrange(
                    "p (r x) -> p r x", x=HW
                )
                pb = ps[(1, c)].rearrange("p (r x) -> p r x", x=HP)[:, :, 0:HW]
                ob = o_sb[1][:, lo_col:hi_col].rearrange(
                    "p (r x) -> p r x", x=HW
                )
                nc.vector.tensor_scalar_add(ob, pb, off_sb)
                (nc.gpsimd if last else nc.sync).dma_start(
                    out_ap[b0 + 1][:, lo_col:hi_col], o_sb[1][:, lo_col:hi_col]
                )
                if last:
                    # Final chunk: VectorE (idle once image B's evict is
                    # done) evicts image A too, so ScalarE's c7 backlog stays
                    # off the tail. Its store goes through the GpSimd SWDGE
                    # queue: the Sync/Scalar teardown drains then don't wait
                    # on this store's ~1.5us HBM write receipt — that wait
                    # moves to the GpSimd drain, which sits right before the
                    # (long) teardown reset block anyway.
                    nc.vector.tensor_scalar_add(oa, pa, off_sb)
                    nc.gpsimd.dma_start(
                        out_ap[b0][:, lo_col:hi_col], o_sb[0][:, lo_col:hi_col]
                    )
                else:
                    nc.scalar.add(oa, pa, off_sb)
                    nc.scalar.dma_start(
                        out_ap[b0][:, lo_col:hi_col], o_sb[0][:, lo_col:hi_col]
                    )


def _build_nc():
    global _NC_CACHE
    if _NC_CACHE is not None:
        return _NC_CACHE
    nc = bacc.Bacc(
        "TRN2", target_bir_lowering=False, debug=False, num_devices=NCORES
    )
    xp_ap = nc.dram_tensor("xp", [BPC, CIN, NPAD], F16, kind="ExternalInput").ap()
    w2_ap = nc.dram_tensor("w2", [128, TAPS, K], F16, kind="ExternalInput").ap()
    off_ap = nc.dram_tensor("off", [K, 1], F32, kind="ExternalInput").ap()
    out_ap = nc.dram_tensor("out", [BPC, K, NOUT], F16, kind="ExternalOutput").ap()
    with tile.TileContext(nc) as tc:
        with ExitStack() as ctx:
            _conv_kernel(ctx, tc, out_ap, xp_ap, w2_ap, off_ap)
    nc.compile()
    _NC_CACHE = nc
    return nc


def _prep_inputs(x, weight, offset):
    """Host-side layout prep: pad x (57-stride grid), transpose+duplicate
    weights, downcast both to fp16."""
    x = np.ascontiguousarray(np.asarray(x, dtype=np.float32))
    weight = np.asarray(weight, dtype=np.float32)
    offset = np.asarray(offset, dtype=np.float32)

    xph = np.zeros((B, CIN, NPAD), dtype=np.float16)
    grid = xph[:, :, BASE : BASE + (HW + 2) * HP].reshape(B, CIN, HW + 2, HP)
    grid[:, :, 1 : 1 + HW, 0:HW] = x.astype(np.float16)

    wt = (
        np.ascontiguousarray(weight.transpose(1, 2, 3, 0))
        .reshape(CIN, TAPS, K)
        .astype(np.float16)
    )
    w2 = np.ascontiguousarray(np.concatenate([wt, wt], axis=0))  # [128, 9, 128]
    off = np.ascontiguousarray(offset.reshape(K, 1))
    return xph, w2, off


def kernel(x, weight, offset):
    nc = _build_nc()
    xph, w2, off = _prep_inputs(x, weight, offset)
    in_maps = [
        {"xp": xph[i * BPC : (i + 1) * BPC], "w2": w2, "off": off}
        for i in range(NCORES)
    ]
    res = run_bass_kernel_spmd(nc, in_maps, list(range(NCORES))).results
    out = np.concatenate(
        [
            res[i]["out"].astype(np.float32).reshape(BPC, K, HW, HW)
            for i in range(NCORES)
        ],
        axis=0,
    )
    return out


# revision 19
# speedup vs baseline: 1.1910x; 1.1910x over previous
"""Trainium2 Bass kernel for nn_ConvLayer: 3x3 conv (stride 1, pad 1) + per-channel offset.

Problem: x[32,64,56,56] (*) w[128,64,3,3] + offset[128,1,1] -> out[32,128,56,56], fp32.

Strategy (8 NeuronCores, data-parallel over batch, 4 images/core):
  - Conv as 9 shifted matmuls (one per 3x3 tap) accumulated in fp32 PSUM.
  - CIN=64 -> each tap is a contract-64 matmul = half the 128x128 PE array.
    Two images are processed CONCURRENTLY via 64x128 row tiling: image A's
    channels live in SBUF partitions 0-63 (PE tile (0,0)), image B's in
    partitions 64-127 (PE tile (64,0)). Each accumulates into its own PSUM
    bank, reaching full PE-array packing (measured 83ns per 399-col dual
    matmul slot = 100% of the 78.6 TF/s roofline).
  - All device-side tensors are fp16: x and weights are downcast on host
    (fp16 products accumulated in fp32 PSUM give ~5e-4 rel err vs the 2e-2
    gate), and the conv output is stored to HBM as fp16 and upcast to fp32
    on host. This halves HBM traffic (10.5 -> 5.1 MB/core) vs fp32.
  - Host pre-pads x to a 57-stride grid: ONE zero column is shared between
    consecutive rows (right-pad of row r == left-pad of row r+1), plus one
    zero row above and below. Every tap read is then a single contiguous
    shifted window. Weights are pre-transposed to [cin, tap, k] (lhsT
    layout) and duplicated into both partition halves.
  - Row-chunk sizes taper: a small first chunk (its input slice lands first
    so the matmul stream starts ASAP) and a small last chunk (so the final
    evict+store+completion tail after the last matmul is short).
  - Output columns are produced on the padded 57-wide grid; the PSUM->SBUF
    eviction (ScalarE for image A, VectorE for image B) compacts to the
    dense 56-wide grid, fuses the per-channel offset add, and downcasts to
    fp16. Each chunk's store is dispatched right after its eviction so the
    output stream never backs up at the end of the kernel.
"""

import numpy as np
from contextlib import ExitStack

import concourse.bass as bass
import concourse.tile as tile
from concourse import bacc, mybir
from concourse.bass_utils import run_bass_kernel_spmd

# Problem constants (hardcoded per contract).
B, CIN, HW, K = 32, 64, 56, 128
NCORES = 8
BPC = B // NCORES          # images per core
HP = HW + 1                # padded row stride: 57 (one shared pad col)
BASE = 1                   # element (row, col) lives at BASE + (row+1)*HP + col
NPAD = BASE + (HW + 2) * HP + 4   # 58 padded rows + tap-read slack: 3312
NOUT = HW * HW             # 3136
TAPS = 9
F16 = mybir.dt.float16
F32 = mybir.dt.float32

# Per-chunk output-row counts (sum 56). Small first chunk -> early start;
# small last chunk -> short tail. 7-row interior chunks (399 cols < 512
# fp32 = one PSUM bank).
RC = [4, 7, 7, 7, 7, 7, 7, 7, 3]
R0 = [sum(RC[:i]) for i in range(len(RC))]  # start row of each chunk
# Chunk groups per PSUM allocation (2 banks per chunk: one per image half).
# Single-chunk first/last groups; two groups in flight (<= 8 banks).
GROUPS = [(0,), (1, 2), (3, 4), (5, 6), (7,), (8,)]

_NC_CACHE = None


def _conv_kernel(ctx: ExitStack, tc: "tile.TileContext", out_ap, xp_ap, w2_ap, off_ap):
    nc = tc.nc
    singles = ctx.enter_context(tc.tile_pool(name="singles", bufs=1))
    xpool = ctx.enter_context(tc.tile_pool(name="xpool", bufs=2))
    opool = ctx.enter_context(tc.tile_pool(name="opool", bufs=2))
    psum = ctx.enter_context(tc.tile_pool(name="psum", bufs=8, space="PSUM"))

    # x-load slices (all on the Sync HWDGE ring, in consumption order).
    # Chunk c's tap reads end at (R0[c]+RC[c]+2)*57 + 2. A small first slice
    # gates chunk 0 so the first matmul starts early; the rest use large
    # per-partition runs for HBM read efficiency.
    gate = [(R0[c] + RC[c] + 2) * HP + 2 for c in range(len(RC))]
    xbounds = [0, gate[0], gate[2], gate[4], gate[6], NPAD]
    # chunks gated by x slices: c0 -> s0; c1,c2 -> s1; c3,c4 -> s2;
    # c5,c6 -> s3; c7,c8 -> s4.

    # Weights as lhsT [c, tap, k], duplicated across both partition halves.
    # One DMA, FIRST in the Sync ring's order: the x slices share that ring
    # and each SDMA engine drains its ring in order, so the weights complete
    # before the x stream — on a separate ring they would crawl at half rate
    # until ~12us (packet round-robin) and stall the first taps.
    # Split into partition halves: each half's descriptors live on a
    # disjoint set of 8 SDMA engines, so a single late-waking engine (a
    # ~2.5us straggler seen on ~1 device per run) only delays one PE half's
    # weights — the other half's matmul stream proceeds.
    w_sb = singles.tile([128, TAPS, K], F16)
    nc.sync.dma_start(w_sb[0:64], w2_ap[0:64])
    nc.sync.dma_start(w_sb[64:128], w2_ap[64:128])
    off_sb = singles.tile([128, 1], F32)
    nc.scalar.dma_start(off_sb[:], off_ap[:])

    # PE warmup: cheap bf16 matmuls on scratch keep TensorE busy through the
    # whole input-DMA head (~4us: body start ~7us to w+x landing ~9-12us,
    # worse when an SDMA engine wakes late). This keeps the HAM activity
    # window saturated (the clock gate opens 1.2 -> 2.4 GHz only after
    # sustained PE activity) and avoids PE-idle gaps on devices whose DMA
    # engines straggle. FULL-ARRAY (128-contract) matmuls: across traces the
    # gate opened ~3.5-6us after the dual-half real stream began but 7-8us
    # of half-array warmups never opened it — the activity monitor appears
    # to weight by array occupancy. ~426ns each at the cold clock.
    scratch = singles.tile([128, 512], mybir.dt.bfloat16)
    nc.vector.memset(scratch[:], 0.0)
    ps_warm = psum.tile([128, 512], F32, tag="ps", name="ps_warm")
    for _ in range(7):
        nc.tensor.matmul(
            ps_warm[:], lhsT=scratch[0:128, 0:128], rhs=scratch[0:128, :],
            start=True, stop=True,
        )

    for pair in range(BPC // 2):
        b0 = 2 * pair
        # Both images of the pair side by side: [2, CIN, NPAD] -> [128, NPAD].
        x_t = xpool.tile([128, NPAD], F16, tag="x")
        xsrc = xp_ap[b0 : b0 + 2].rearrange("b c n -> (b c) n")
        for s in range(len(xbounds) - 1):
            lo, hi = xbounds[s], xbounds[s + 1]
            if s == 0 and pair == 0:
                # First slice gates the very first matmuls: split into
                # partition halves (disjoint SDMA-engine sets) so a
                # late-waking engine only stalls one PE half.
                nc.sync.dma_start(x_t[0:64, lo:hi], xsrc[0:64, lo:hi])
                nc.sync.dma_start(x_t[64:128, lo:hi], xsrc[64:128, lo:hi])
            else:
                nc.sync.dma_start(x_t[:, lo:hi], xsrc[:, lo:hi])
        o_sb = [
            opool.tile([128, NOUT], F16, tag="oA", name=f"oA_{pair}"),
            opool.tile([128, NOUT], F16, tag="oB", name=f"oB_{pair}"),
        ]

        for grp in GROUPS:
            ps = {}
            for half in (0, 1):
                for c in grp:
                    ps[(half, c)] = psum.tile(
                        [128, RC[c] * HP], F32, tag="ps",
                        name=f"ps_{pair}_{half}_{c}",
                    )
            for t in range(TAPS):
                kh, kw = divmod(t, 3)
                o = kh * HP + kw
                st, sp = (t == 0), (t == TAPS - 1)
                for half in (0, 1):
                    lo, hi = 64 * half, 64 * half + 64
                    for c in grp:
                        base = R0[c] * HP + o
                        nc.tensor.matmul(
                            ps[(half, c)][:],
                            lhsT=w_sb[lo:hi, t, :],
                            rhs=x_t[lo:hi, base : base + RC[c] * HP],
                            start=st,
                            stop=sp,
                        )
            # Evict: compact 57-stride padded rows to 56-wide dense rows, add
            # the per-channel offset, downcast to fp16. Image A on ScalarE,
            # image B on VectorE (they hit different PSUM banks in parallel).
            # Each chunk's store is dispatched right after its eviction so the
            # output stream never builds an end-of-kernel backlog: the final
            # chunk's store is the only transfer left after the last matmul.
            # Image A rides the Scalar HWDGE ring, image B the Sync ring.
            for c in grp:
                last = c == len(RC) - 1
                lo_col, hi_col = R0[c] * HW, (R0[c] + RC[c]) * HW
                pa = ps[(0, c)].rearrange("p (r x) -> p r x", x=HP)[:, :, 0:HW]
                oa = o_sb[0][:, lo_col:hi_col].rearrange(
                    "p (r x) -> p r x", x=HW
                )
                pb = ps[(1, c)].rearrange("p (r x) -> p r x", x=HP)[:, :, 0:HW]
                ob = o_sb[1][:, lo_col:hi_col].rearrange(
                    "p (r x) -> p r x", x=HW
                )
                nc.vector.tensor_scalar_add(ob, pb, off_sb)
                nc.sync.dma_start(
                    out_ap[b0 + 1][:, lo_col:hi_col], o_sb[1][:, lo_col:hi_col]
                )
                if last:
                    # Final chunk: VectorE (idle once image B's evict is
                    # done) evicts image A too, so ScalarE's c7 backlog stays
                    # off the tail. The two stores go on DIFFERENT engines
                    # (B on Sync above, A on Scalar here) so their dispatches
                    # and ring transfers run in parallel.
                    nc.vector.tensor_scalar_add(oa, pa, off_sb)
                    nc.scalar.dma_start(
                        out_ap[b0][:, lo_col:hi_col], o_sb[0][:, lo_col:hi_col]
                    )
                else:
                    nc.scalar.add(oa, pa, off_sb)
                    nc.scalar.dma_start(
                        out_ap[b0][:, lo_col:hi_col], o_sb[0][:, lo_col:hi_col]
                    )


def _build_nc():
    global _NC_CACHE
    if _NC_CACHE is not None:
        return _NC_CACHE
    nc = bacc.Bacc(
        "TRN2", target_bir_lowering=False, debug=False, num_devices=NCORES
    )
    xp_ap = nc.dram_tensor("xp", [BPC, CIN, NPAD], F16, kind="ExternalInput").ap()
    w2_ap = nc.dram_tensor("w2", [128, TAPS, K], F16, kind="ExternalInput").ap()
    off_ap = nc.dram_tensor("off", [K, 1], F32, kind="ExternalInput").ap()
    out_ap = nc.dram_tensor("out", [BPC, K, NOUT], F16, kind="ExternalOutput").ap()
    with tile.TileContext(nc) as tc:
        with ExitStack() as ctx:
            _conv_kernel(ctx, tc, out_ap, xp_ap, w2_ap, off_ap)
    nc.compile()
    _NC_CACHE = nc
    return nc


def _prep_inputs(x, weight, offset):
    """Host-side layout prep: pad x (57-stride grid), transpose+duplicate
    weights, downcast both to fp16."""
    x = np.ascontiguousarray(np.asarray(x, dtype=np.float32))
    weight = np.asarray(weight, dtype=np.float32)
    offset = np.asarray(offset, dtype=np.float32)

    xph = np.zeros((B, CIN, NPAD), dtype=np.float16)
    grid = xph[:, :, BASE : BASE + (HW + 2) * HP].reshape(B, CIN, HW + 2, HP)
    grid[:, :, 1 : 1 + HW, 0:HW] = x.astype(np.float16)

    wt = (
        np.ascontiguousarray(weight.transpose(1, 2, 3, 0))
        .reshape(CIN, TAPS, K)
        .astype(np.float16)
    )
    w2 = np.ascontiguousarray(np.concatenate([wt, wt], axis=0))  # [128, 9, 128]
    off = np.ascontiguousarray(offset.reshape(K, 1))
    return xph, w2, off


def kernel(x, weight, offset):
    nc = _build_nc()
    xph, w2, off = _prep_inputs(x, weight, offset)
    in_maps = [
        {"xp": xph[i * BPC : (i + 1) * BPC], "w2": w2, "off": off}
        for i in range(NCORES)
    ]
    res = run_bass_kernel_spmd(nc, in_maps, list(range(NCORES))).results
    out = np.concatenate(
        [
            res[i]["out"].astype(np.float32).reshape(BPC, K, HW, HW)
            for i in range(NCORES)
        ],
        axis=0,
    )
    return out
